# revision 9
# baseline (speedup 1.0000x reference)
"""ContrastiveLoss (nn_ContrastiveLoss_17093969838495) Trainium2 kernel.

Math: for p1, p2 in R^{BxD} the reference computes
    pos_loss = sum((p1-p2)^2)/B
    d[i,j]   = ||p1_i||^2 + ||p2_j||^2 - 2 <p1_i, p2_j>
    neg_loss = -(sum(d) - trace(d)) / (B*(B-1))
    out      = pos_loss + neg_loss

The sum-of-squares terms cancel exactly:
    out = -2*P/(B-1) + 2*G/(B*(B-1))
where P = sum(p1 * p2) and G = colsum(p1) . colsum(p2).

So each core only needs, over its 512-row block:
  - P partials: per-span DVE multiply with fused accumulate (accum_out)
  - column sums: per-(row-tile, 128-column-chunk) one-shot PE matmuls
    against a ones vector (PSUM accumulation across tiles is numerically
    broken when groups interleave on this toolchain), folded over the 4
    row-tiles by DVE tensor_reduce passes scheduled into the vector
    engine's idle gaps.
The kernel is input-DMA bound (16.8 MB/core ~ 46.6 us at 360 GB/s model
bandwidth); both input pools are fully resident (bufs=4) so every DMA
front-end runs early and the 18 input transfers pack back-to-back.  The
trailing row-tile is DMA'd in tapered column spans (2048,1024,512,256,
128,128) sized so the vector engine goes idle exactly when each span's
900ns-delayed completion semaphore fires.

Writeback is split so almost nothing rides the post-last-byte chain:
  DMA-X: chunk 0-15 colsums (folded after the 2048-span product) + early
         product accumulators; its descriptor generation overlaps the
         input stream and its transfer queues FIFO behind all inputs.
  DMA-Y: chunk 16-30 colsums (folded in the DVE gap before the last
         product), chunk 31's EIGHT raw per-tile PSUM columns (copied by
         the otherwise-idle ACT engine, folded on host), late products.
Host combines the 8 per-core [128, 79] partials in float64.
"""

import numpy as np

try:
    import concourse.bass as bass
except ImportError:  # pragma: no cover - path fallback for fresh dirs
    import sys

    sys.path.insert(0, "/opt/trn_rl_repo")
    import concourse.bass as bass

import concourse.bacc as bacc
import concourse.tile as tile
from concourse import mybir
from concourse.bass_utils import run_bass_kernel_spmd

N_CORES = 8
B = 4096
D = 4096
RB = B // N_CORES  # 512 rows per core
P = 128  # SBUF partitions
NT = RB // P  # 4 row-tiles per core
NCH = D // P  # 32 column chunks of 128
# DMA span widths per row-tile (tapered tail, see module docstring)
SPANS = ((4096,), (4096,), (4096,), (2048, 1024, 512, 256, 128, 128))
A_CH = 16  # chunks 0..15   -> fold-A, ships on DMA-X
B_CH = 31  # chunks 16..30  -> fold-B, ships on DMA-Y
# out_sb column layout:
#   [0:32]   chunk 0..15 colsums, pair-interleaved (2j = p1, 2j+1 = p2)
#   [32:36]  early product accumulators (t0, t1, t2, t3-2048 spans)
#   [36:66]  chunk 16..30 colsums, pair-interleaved
#   [66:74]  chunk 31 raw per-tile colsums (66+2t = p1 tile t, 67+2t = p2)
#   [74:79]  late product accumulators (t3 spans 1024, 512, 256, 128, 128)
X_COLS = 36
OUT_COLS = 79
PROD_COLS = ((32,), (33,), (34,), (35, 74, 75, 76, 77, 78))

_CACHE = {}


def build_program(replicas=1):
    f32 = mybir.dt.float32
    nc = bacc.Bacc(
        "TRN2", target_bir_lowering=False, debug=False, num_devices=N_CORES
    )
    p1 = nc.dram_tensor("p1", [RB, D], f32, kind="ExternalInput")
    p2 = nc.dram_tensor("p2", [RB, D], f32, kind="ExternalInput")
    out = nc.dram_tensor("out", [P, OUT_COLS], f32, kind="ExternalOutput")

    with tile.TileContext(nc) as tc:
        with (
            tc.tile_pool(name="in1", bufs=NT) as pool1,
            tc.tile_pool(name="in2", bufs=NT) as pool2,
            tc.tile_pool(name="scr", bufs=2) as scrp,
            tc.tile_pool(name="misc", bufs=1) as misc,
            tc.tile_pool(name="outp", bufs=1) as outp,
            tc.tile_pool(name="psum", bufs=1, space=bass.MemorySpace.PSUM) as psp,
        ):
            ones = misc.tile([P, 1], f32)
            nc.vector.memset(ones[:], 1.0)
            for _rep in range(replicas):
                _build_body(nc, pool1, pool2, scrp, outp, psp, ones, p1, p2, out)

    nc.compile()
    return nc


def _build_body(nc, pool1, pool2, scrp, outp, psp, ones, p1, p2, out):
    f32 = mybir.dt.float32
    ax = mybir.AxisListType.X
    add = mybir.AluOpType.add
    out_sb = outp.tile([P, OUT_COLS], f32, tag="out_sb")
    # per-(tile, chunk) one-shot column sums; [p, t, 2jj(+1)] = tile t,
    # chunk-group-local index jj, p1/p2 pair-interleaved.  Separate tiles per
    # fold group so each fold's RAW dependency covers only its own chunks.
    cs_a = psp.tile([P, NT, 2 * A_CH], f32, tag="cs_a")
    cs_b = psp.tile([P, NT, 2 * (B_CH - A_CH)], f32, tag="cs_b")
    cs_l = psp.tile([P, NT, 2], f32, tag="cs_l")

    def fold(cst, dst):  # reduce over the row-tile axis (DVE)
        nc.vector.tensor_reduce(
            out=dst, in_=cst[:].rearrange("p t j -> p j t"), axis=ax, op=add
        )

    for t in range(NT):
        rows = slice(t * P, (t + 1) * P)
        p1t = pool1.tile([P, D], f32, tag="p1t")
        p2t = pool2.tile([P, D], f32, tag="p2t")
        off = 0
        for si, cw in enumerate(SPANS[t]):
            sl = slice(off, off + cw)
            nc.sync.dma_start(out=p2t[:, sl], in_=p2[rows, sl])
            nc.sync.dma_start(out=p1t[:, sl], in_=p1[rows, sl])

            # sum(p1*p2) per partition (DVE, fused multiply+accumulate;
            # tensor_tensor_reduce crashes on this HW/toolchain)
            pc = PROD_COLS[t][si]
            s3 = scrp.tile([P, D], f32, tag="scr")
            nc.vector.scalar_tensor_tensor(
                out=s3[:, 0:cw],
                in0=p1t[:, sl],
                scalar=1.0,
                in1=p2t[:, sl],
                op0=mybir.AluOpType.mult,
                op1=mybir.AluOpType.mult,
                accum_out=out_sb[:, pc : pc + 1],
            )

            # column sums via PE: cs[m, t, jj] = sum_rows p_t[:, j*128+m]
            for j in range(off // P, (off + cw) // P):
                if j < A_CH:
                    cst, jj = cs_a, j
                elif j < B_CH:
                    cst, jj = cs_b, j - A_CH
                else:
                    cst, jj = cs_l, 0
                nc.tensor.matmul(
                    cst[:, t, 2 * jj : 2 * jj + 1],
                    p1t[:, j * P : (j + 1) * P],
                    ones[:],
                )
                nc.tensor.matmul(
                    cst[:, t, 2 * jj + 1 : 2 * jj + 2],
                    p2t[:, j * P : (j + 1) * P],
                    ones[:],
                )
            off += cw

            if t == NT - 1 and si == 0:
                # fold-A lands in the DVE gap after this span's product
                fold(cs_a, out_sb[:, 0 : 2 * A_CH])
            if t == NT - 1 and si == len(SPANS[t]) - 2:
                # fold-B fits the DVE idle gap between the last two span
                # products (gated on chunk 30's final matmul)
                fold(cs_b, out_sb[:, X_COLS : X_COLS + 2 * (B_CH - A_CH)])

    # Everything DMA-X carries is final ~2us before the last input byte, so
    # its descriptor generation overlaps the input stream and its transfer
    # queues FIFO behind all input transfers, landing in the idle window at
    # the end.  Placed after the span loop so its SEQ-held semaphore wait
    # cannot stall later input-DMA front-ends.
    nc.sync.dma_start(out=out[:, 0:X_COLS], in_=out_sb[:, 0:X_COLS])

    # chunk 31's raw per-tile columns: copied by the idle ACT engine right
    # after the final two matmuls, folded on host
    nc.scalar.copy(out_sb[:, 66:74], cs_l[:])
    nc.sync.dma_start(out=out[:, X_COLS:OUT_COLS], in_=out_sb[:, X_COLS:OUT_COLS])


def _get_program():
    if "nc" not in _CACHE:
        _CACHE["nc"] = build_program()
    return _CACHE["nc"]


def run_device(p1, p2, trace=False):
    """Run the SPMD kernel; returns (per-core outs list, BassKernelResults)."""
    nc = _get_program()
    in_maps = [
        {
            "p1": np.ascontiguousarray(p1[c * RB : (c + 1) * RB]),
            "p2": np.ascontiguousarray(p2[c * RB : (c + 1) * RB]),
        }
        for c in range(N_CORES)
    ]
    try:
        bres = run_bass_kernel_spmd(nc, in_maps, list(range(N_CORES)), trace=trace)
    except ModuleNotFoundError:
        # axon NTFF profile hook unavailable in this image; run untraced
        import os

        os.environ["BASS_NEVER_TRACE"] = "1"
        bres = run_bass_kernel_spmd(nc, in_maps, list(range(N_CORES)), trace=False)
    except Exception:
        # transient device wedge (NRT_EXEC_UNIT_UNRECOVERABLE) recovers after
        # a short wait; retry once before giving up
        import time

        time.sleep(30)
        bres = run_bass_kernel_spmd(nc, in_maps, list(range(N_CORES)), trace=False)
    return [r["out"] for r in bres.results], bres


def combine_partials(outs):
    """float64 combine of the per-core [P, OUT_COLS] partials -> f32 scalar."""
    total = np.zeros((P, OUT_COLS), np.float64)
    for o in outs:
        total += o.astype(np.float64)

    s1 = np.empty(D, np.float64)  # colsum(p1), index j*128+m
    s2 = np.empty(D, np.float64)  # colsum(p2)
    for j in range(NCH):
        if j < A_CH:
            b1, b2 = 2 * j, 2 * j + 1
        elif j < B_CH:
            b1, b2 = X_COLS + 2 * (j - A_CH), X_COLS + 2 * (j - A_CH) + 1
        else:  # chunk 31: fold the 4 per-tile raw columns here
            s1[j * P : (j + 1) * P] = total[:, 66:74:2].sum(axis=1)
            s2[j * P : (j + 1) * P] = total[:, 67:74:2].sum(axis=1)
            continue
        s1[j * P : (j + 1) * P] = total[:, b1]
        s2[j * P : (j + 1) * P] = total[:, b2]
    pp = total[:, 32:36].sum() + total[:, 74:79].sum()  # sum(p1 * p2)

    G = s1 @ s2  # sum of the full Gram matrix
    result = -2.0 * pp / (B - 1) + 2.0 * G / (B * (B - 1))
    return np.asarray(result, dtype=np.float32)


def kernel(postive1, postive2):
    p1 = np.ascontiguousarray(np.asarray(postive1, dtype=np.float32))
    p2 = np.ascontiguousarray(np.asarray(postive2, dtype=np.float32))
    assert p1.shape == (B, D) and p2.shape == (B, D)
    outs, _ = run_device(p1, p2, trace=False)
    return combine_partials(outs)


# revision 12
# speedup vs baseline: 1.0002x; 1.0002x over previous
"""ContrastiveLoss (nn_ContrastiveLoss_17093969838495) Trainium2 kernel.

Math: for p1, p2 in R^{BxD} the reference computes
    pos_loss = sum((p1-p2)^2)/B
    d[i,j]   = ||p1_i||^2 + ||p2_j||^2 - 2 <p1_i, p2_j>
    neg_loss = -(sum(d) - trace(d)) / (B*(B-1))
    out      = pos_loss + neg_loss

The sum-of-squares terms cancel exactly:
    out = -2*P/(B-1) + 2*G/(B*(B-1))
where P = sum(p1 * p2) and G = colsum(p1) . colsum(p2).

So each core only needs, over its 512-row block:
  - P partials: per-span DVE multiply with fused accumulate (accum_out)
  - column sums: per-(row-tile, 128-column-chunk) one-shot PE matmuls
    against a ones vector (PSUM accumulation across tiles is numerically
    broken when groups interleave on this toolchain), folded over the 4
    row-tiles by DVE tensor_reduce passes scheduled into the vector
    engine's idle gaps.
The kernel is input-DMA bound (16.8 MB/core ~ 46.6 us at 360 GB/s model
bandwidth); both input pools are fully resident (bufs=4) so every DMA
front-end runs early and the 18 input transfers pack back-to-back.  The
trailing row-tile is DMA'd in tapered column spans (2048,1024,512,256,
128,128) sized so the vector engine goes idle exactly when each span's
900ns-delayed completion semaphore fires.

Writeback is split so almost nothing rides the post-last-byte chain:
  DMA-X: chunk 0-15 colsums (folded after the 2048-span product) + early
         product accumulators; its descriptor generation overlaps the
         input stream and its transfer queues FIFO behind all inputs.
  DMA-Y: chunk 16-30 colsums (folded in the DVE gap before the last
         product), chunk 31's EIGHT raw per-tile PSUM columns (copied by
         the otherwise-idle ACT engine, folded on host), late products.
Host combines the 8 per-core [128, 79] partials in float64.
"""

import numpy as np

try:
    import concourse.bass as bass
except ImportError:  # pragma: no cover - path fallback for fresh dirs
    import sys

    sys.path.insert(0, "/opt/trn_rl_repo")
    import concourse.bass as bass

import concourse.bacc as bacc
import concourse.tile as tile
from concourse import mybir
from concourse.bass_utils import run_bass_kernel_spmd

N_CORES = 8
B = 4096
D = 4096
RB = B // N_CORES  # 512 rows per core
P = 128  # SBUF partitions
NT = RB // P  # 4 row-tiles per core
NCH = D // P  # 32 column chunks of 128
# DMA span widths per row-tile (tapered tail, see module docstring)
SPANS = ((4096,), (4096,), (4096,), (2048, 1024, 512, 256, 128, 128))
A_CH = 16  # chunks 0..15   -> fold-A, ships on DMA-X
B_CH = 31  # chunks 16..30  -> fold-B, ships on DMA-Y
# out_sb column layout:
#   [0:32]   chunk 0..15 colsums, pair-interleaved (2j = p1, 2j+1 = p2)
#   [32:39]  product accumulators done early enough for DMA-X (t0, t1, t2,
#            t3 spans 2048, 1024, 512, 256 -- the 256-span product's
#            semaphore fires just early enough that DMA-X's descriptor
#            generation still clears the shared HWDGE before DMA-Y needs it)
#   [39:69]  chunk 16..30 colsums, pair-interleaved
#   [69:77]  chunk 31 raw per-tile colsums (69+2t = p1 tile t, 70+2t = p2)
#   [77:79]  late product accumulators (t3 spans 128, 128)
X_COLS = 39
OUT_COLS = 79
PROD_COLS = ((32,), (33,), (34,), (35, 36, 37, 38, 77, 78))

_CACHE = {}


def build_program(replicas=1):
    f32 = mybir.dt.float32
    nc = bacc.Bacc(
        "TRN2", target_bir_lowering=False, debug=False, num_devices=N_CORES
    )
    p1 = nc.dram_tensor("p1", [RB, D], f32, kind="ExternalInput")
    p2 = nc.dram_tensor("p2", [RB, D], f32, kind="ExternalInput")
    out = nc.dram_tensor("out", [P, OUT_COLS], f32, kind="ExternalOutput")

    with tile.TileContext(nc) as tc:
        with (
            tc.tile_pool(name="in1", bufs=NT) as pool1,
            tc.tile_pool(name="in2", bufs=NT) as pool2,
            tc.tile_pool(name="scr", bufs=2) as scrp,
            tc.tile_pool(name="misc", bufs=1) as misc,
            tc.tile_pool(name="outp", bufs=1) as outp,
            tc.tile_pool(name="psum", bufs=1, space=bass.MemorySpace.PSUM) as psp,
        ):
            ones = misc.tile([P, 1], f32)
            nc.vector.memset(ones[:], 1.0)
            for _rep in range(replicas):
                _build_body(nc, pool1, pool2, scrp, outp, psp, ones, p1, p2, out)

    nc.compile()
    return nc


def _build_body(nc, pool1, pool2, scrp, outp, psp, ones, p1, p2, out):
    f32 = mybir.dt.float32
    ax = mybir.AxisListType.X
    add = mybir.AluOpType.add
    out_sb = outp.tile([P, OUT_COLS], f32, tag="out_sb")
    # per-(tile, chunk) one-shot column sums; [p, t, 2jj(+1)] = tile t,
    # chunk-group-local index jj, p1/p2 pair-interleaved.  Separate tiles per
    # fold group so each fold's RAW dependency covers only its own chunks.
    cs_a = psp.tile([P, NT, 2 * A_CH], f32, tag="cs_a")
    cs_b = psp.tile([P, NT, 2 * (B_CH - A_CH)], f32, tag="cs_b")
    cs_l = psp.tile([P, NT, 2], f32, tag="cs_l")

    def fold(cst, dst):  # reduce over the row-tile axis (DVE)
        nc.vector.tensor_reduce(
            out=dst, in_=cst[:].rearrange("p t j -> p j t"), axis=ax, op=add
        )

    for t in range(NT):
        rows = slice(t * P, (t + 1) * P)
        p1t = pool1.tile([P, D], f32, tag="p1t")
        p2t = pool2.tile([P, D], f32, tag="p2t")
        off = 0
        for si, cw in enumerate(SPANS[t]):
            sl = slice(off, off + cw)
            nc.sync.dma_start(out=p2t[:, sl], in_=p2[rows, sl])
            nc.sync.dma_start(out=p1t[:, sl], in_=p1[rows, sl])

            # sum(p1*p2) per partition (DVE, fused multiply+accumulate;
            # tensor_tensor_reduce crashes on this HW/toolchain)
            pc = PROD_COLS[t][si]
            s3 = scrp.tile([P, D], f32, tag="scr")
            nc.vector.scalar_tensor_tensor(
                out=s3[:, 0:cw],
                in0=p1t[:, sl],
                scalar=1.0,
                in1=p2t[:, sl],
                op0=mybir.AluOpType.mult,
                op1=mybir.AluOpType.mult,
                accum_out=out_sb[:, pc : pc + 1],
            )

            # column sums via PE: cs[m, t, jj] = sum_rows p_t[:, j*128+m]
            for j in range(off // P, (off + cw) // P):
                if j < A_CH:
                    cst, jj = cs_a, j
                elif j < B_CH:
                    cst, jj = cs_b, j - A_CH
                else:
                    cst, jj = cs_l, 0
                nc.tensor.matmul(
                    cst[:, t, 2 * jj : 2 * jj + 1],
                    p1t[:, j * P : (j + 1) * P],
                    ones[:],
                )
                nc.tensor.matmul(
                    cst[:, t, 2 * jj + 1 : 2 * jj + 2],
                    p2t[:, j * P : (j + 1) * P],
                    ones[:],
                )
            off += cw

            if t == NT - 1 and si == 0:
                # fold-A lands in the DVE gap after this span's product
                fold(cs_a, out_sb[:, 0 : 2 * A_CH])
            if t == NT - 1 and si == len(SPANS[t]) - 2:
                # fold-B fits the DVE idle gap between the last two span
                # products (gated on chunk 30's final matmul)
                fold(cs_b, out_sb[:, X_COLS : X_COLS + 2 * (B_CH - A_CH)])

    # Everything DMA-X carries is final ~2us before the last input byte, so
    # its descriptor generation overlaps the input stream and its transfer
    # queues FIFO behind all input transfers, landing in the idle window at
    # the end.  Placed after the span loop so its SEQ-held semaphore wait
    # cannot stall later input-DMA front-ends.
    nc.sync.dma_start(out=out[:, 0:X_COLS], in_=out_sb[:, 0:X_COLS])

    # chunk 31's raw per-tile columns: copied by the idle ACT engine right
    # after the final two matmuls, folded on host
    nc.scalar.copy(out_sb[:, 69:77], cs_l[:])
    nc.sync.dma_start(out=out[:, X_COLS:OUT_COLS], in_=out_sb[:, X_COLS:OUT_COLS])


def _get_program():
    if "nc" not in _CACHE:
        _CACHE["nc"] = build_program()
    return _CACHE["nc"]


def run_device(p1, p2, trace=False):
    """Run the SPMD kernel; returns (per-core outs list, BassKernelResults)."""
    nc = _get_program()
    in_maps = [
        {
            "p1": np.ascontiguousarray(p1[c * RB : (c + 1) * RB]),
            "p2": np.ascontiguousarray(p2[c * RB : (c + 1) * RB]),
        }
        for c in range(N_CORES)
    ]
    try:
        bres = run_bass_kernel_spmd(nc, in_maps, list(range(N_CORES)), trace=trace)
    except ModuleNotFoundError:
        # axon NTFF profile hook unavailable in this image; run untraced
        import os

        os.environ["BASS_NEVER_TRACE"] = "1"
        bres = run_bass_kernel_spmd(nc, in_maps, list(range(N_CORES)), trace=False)
    except Exception:
        # transient device wedge (NRT_EXEC_UNIT_UNRECOVERABLE) recovers after
        # a short wait; retry once before giving up
        import time

        time.sleep(30)
        bres = run_bass_kernel_spmd(nc, in_maps, list(range(N_CORES)), trace=False)
    return [r["out"] for r in bres.results], bres


def combine_partials(outs):
    """float64 combine of the per-core [P, OUT_COLS] partials -> f32 scalar."""
    total = np.zeros((P, OUT_COLS), np.float64)
    for o in outs:
        total += o.astype(np.float64)

    s1 = np.empty(D, np.float64)  # colsum(p1), index j*128+m
    s2 = np.empty(D, np.float64)  # colsum(p2)
    for j in range(NCH):
        if j < A_CH:
            b1, b2 = 2 * j, 2 * j + 1
        elif j < B_CH:
            b1, b2 = X_COLS + 2 * (j - A_CH), X_COLS + 2 * (j - A_CH) + 1
        else:  # chunk 31: fold the 4 per-tile raw columns here
            s1[j * P : (j + 1) * P] = total[:, 69:77:2].sum(axis=1)
            s2[j * P : (j + 1) * P] = total[:, 70:77:2].sum(axis=1)
            continue
        s1[j * P : (j + 1) * P] = total[:, b1]
        s2[j * P : (j + 1) * P] = total[:, b2]
    pp = total[:, 32:39].sum() + total[:, 77:79].sum()  # sum(p1 * p2)

    G = s1 @ s2  # sum of the full Gram matrix
    result = -2.0 * pp / (B - 1) + 2.0 * G / (B * (B - 1))
    return np.asarray(result, dtype=np.float32)


def kernel(postive1, postive2):
    p1 = np.ascontiguousarray(np.asarray(postive1, dtype=np.float32))
    p2 = np.ascontiguousarray(np.asarray(postive2, dtype=np.float32))
    assert p1.shape == (B, D) and p2.shape == (B, D)
    outs, _ = run_device(p1, p2, trace=False)
    return combine_partials(outs)
